# revision 1
# baseline (speedup 1.0000x reference)
"""Trainium2 Bass kernel: GNN message-passing layer.

Computes, for a graph with E=100000 edges and A=20000 atoms (D=64):
    sent     = atom_matrix[connectivity[:, 1]]          # (E, D) gather
    messages = einsum('eij,ej->ei', bond_matrix, sent)  # per-edge matvec
    out      = segment_sum(messages, connectivity[:, 0], A)  # sorted ids

Sharding: edges are split contiguously across 8 NeuronCores (12500 each).
The host performs the gather (pure indexing) and hands each core its
bond-matrix shard plus the gathered sending-atom states.  bond_matrix
dominates traffic (1.6 GB, ~205 MB/core); the per-core HBM limit is
~358 GB/s, so the kernel is DMA-bound at ~570 us/core.  The compute has
to keep up with that rate, which a single engine cannot do in fp32, so
work is split between two paths (statically, identical on every core —
SPMD runs one program):

* Every 128-edge tile: DVE computes P[e,i,j] = B[e,i,j] * x[e,j] with a
  single tensor_tensor whose x operand has a 0-step broadcast AP dim.
* PE-path (superblocks of 4 consecutive tiles): the TensorEngine
  contracts the *edge* dimension against a host-built one-hot matrix
  S[e, a-a0] while streaming P un-reduced: psum[a-a0, (i,j)] += S^T @ P
  (4096 psum columns = the whole PSUM).  One DVE grouped reduce per
  superblock then folds j: out[a-a0, i] = sum_j psum.  This replaces a
  4096-element DVE reduce per tile with one per 4 tiles.
* V-path (remaining tiles): DVE grouped tensor_reduce produces per-edge
  messages which return to the host.

The host combines: PE window partials are added at their atom offsets,
V messages go through a sorted segment-sum (np.add.reduceat), and the
rare edges whose receiving atom falls >=128 atoms past their
superblock's first atom ("overflow") are recomputed on the host.
"""

import os
import numpy as np

import concourse.bass as bass
import concourse.bacc as bacc
import concourse.mybir as mybir
import concourse.tile as tile
from concourse import bass_utils

N_ATOMS = 20000
N_EDGES = 100000
D = 64
DD = D * D
NCORES = 8
E_PER = N_EDGES // NCORES        # 12500 edges per core
TILE_E = 128                     # edges per SBUF tile (partition dim)
NT_FULL = E_PER // TILE_E        # 97 full tiles
TAIL = E_PER - NT_FULL * TILE_E  # 84 edges in the tail tile
G_SB = 4                         # tiles per PE superblock

# Tuning knobs ---------------------------------------------------------------
# PSUM_TRICK: matmul output AP repeats each psum address across the 64 j
# columns (0-step free dim); the PE's has_written accumulate folds the
# j-reduction into PSUM, so psum is [128, 64], no DVE flush is needed, and
# superblocks collapse to single tiles (g_sb=1).
PSUM_TRICK = os.environ.get("KERNEL_PSUM_TRICK", "1") == "1"
K_PE = int(os.environ.get("KERNEL_K_PE", "76" if PSUM_TRICK else "21"))
MM_DTYPE = os.environ.get("KERNEL_MM_DTYPE", "fp32")  # fp32 | fp32r | bf16
G_SB_EFF = 1 if PSUM_TRICK else G_SB
# ---------------------------------------------------------------------------

F32 = mybir.dt.float32
F32R = mybir.dt.float32r
BF16 = mybir.dt.bfloat16

LAST_RESULTS = None
_NC_CACHE = {}


def _schedule(nt_full=NT_FULL, tail=TAIL, k_pe=K_PE, g_sb=None):
    if g_sb is None:
        g_sb = G_SB_EFF
    """Static tile schedule, identical on every core.

    Returns a list of entries:
      ("pe", [t0, .., t_{g-1}])  - superblock of consecutive full tiles
      ("v", t)                   - V-path full tile
      ("tail", nt_full)          - the partial tail tile (V-path)
    Tile t covers edges [t*128, (t+1)*128) of the core's shard.
    """
    k_pe = min(k_pe, nt_full // g_sb)
    sched = []
    t = 0
    n_v = nt_full - k_pe * g_sb
    # interleave: after each superblock, emit roughly n_v/k_pe V tiles
    v_emitted = 0
    for s in range(k_pe):
        sched.append(("pe", list(range(t, t + g_sb))))
        t += g_sb
        want = int(round((s + 1) * n_v / max(k_pe, 1)))
        while v_emitted < want:
            sched.append(("v", t))
            t += 1
            v_emitted += 1
    while t < nt_full:
        sched.append(("v", t))
        t += 1
    if tail:
        sched.append(("tail", nt_full))
    return sched


def _mm_dts():
    """(S dtype, P dtype) for the PE-path matmul operands.

    fp32r: the BIR verifier requires matmul operands to be *produced* as
    rounded fp32r (a bitcast of raw fp32 is rejected), so the S dram tensor
    and the DVE-mul output tile are declared float32r end-to-end.
    """
    if MM_DTYPE == "fp32r":
        return F32R, F32R
    if MM_DTYPE == "bf16":
        return BF16, BF16
    return F32, F32


def _build_nc(nt_full=NT_FULL, tail=TAIL, k_pe=K_PE, reps=1):
    """Build the single-core Bass program (same program on all cores).

    reps > 1 wraps the whole schedule in a device-side For_i loop (used for
    benchmarking: amortizes host dispatch overhead out of the measurement).
    """
    sched = _schedule(nt_full, tail, k_pe)
    n_pe_tiles = sum(len(e[1]) for e in sched if e[0] == "pe")
    n_sb = sum(1 for e in sched if e[0] == "pe")
    n_v = sum(1 for e in sched if e[0] == "v")
    s_dt, p_dt = _mm_dts()

    nc = bacc.Bacc("TRN2", target_bir_lowering=False, debug=False)

    bond_m = nc.dram_tensor("bond_m", [nt_full, TILE_E, DD], F32,
                            kind="ExternalInput")
    xs_m = nc.dram_tensor("xs_m", [nt_full, TILE_E, D], F32,
                          kind="ExternalInput")
    if n_pe_tiles:
        s_d = nc.dram_tensor("s_onehot", [n_pe_tiles, TILE_E, TILE_E], s_dt,
                             kind="ExternalInput")
        out_pe = nc.dram_tensor("out_pe", [n_sb, TILE_E, D], F32,
                                kind="ExternalOutput")
    if n_v:
        msg_v = nc.dram_tensor("msg_v", [n_v, TILE_E, D], F32,
                               kind="ExternalOutput")
    if tail:
        bond_t = nc.dram_tensor("bond_t", [tail, DD], F32,
                                kind="ExternalInput")
        xs_t = nc.dram_tensor("xs_t", [tail, D], F32, kind="ExternalInput")
        msg_t = nc.dram_tensor("msg_t", [tail, D], F32, kind="ExternalOutput")

    with tile.TileContext(nc) as tc:
        from contextlib import ExitStack
        with tc.tile_pool(name="bp", bufs=4) as bp, \
             tc.tile_pool(name="xp", bufs=6) as xp, \
             tc.tile_pool(name="pp", bufs=4) as pp, \
             tc.tile_pool(name="sp", bufs=4) as sp, \
             tc.tile_pool(name="mp", bufs=6) as mp, \
             tc.tile_pool(name="op", bufs=4) as op, \
             tc.tile_pool(name="ps", bufs=(8 if PSUM_TRICK else 1),
                          space=bass.MemorySpace.PSUM) as ps, \
             ExitStack() as loop_ctx:
            if reps > 1:
                loop_ctx.enter_context(tc.For_i(0, reps, 1))

            pe_i = 0   # PE tile counter (indexes s_onehot)
            sb_i = 0   # superblock counter (indexes out_pe)
            v_i = 0    # V tile counter (indexes msg_v)

            def load_and_mul(t, is_tail=False, pe=False):
                """DMA bond+x for tile t, return P tile.

                PE-path tiles produce dtype p_dt (fp32r/bf16 when the
                streaming matmul needs it); V-path tiles always stay fp32
                and multiply in place (P overwrites the bond tile)."""
                n = tail if is_tail else TILE_E
                bsrc = bond_t[:] if is_tail else bond_m[t]
                xsrc = xs_t[:] if is_tail else xs_m[t]
                bt = bp.tile([TILE_E, DD], F32, tag="b")
                nc.sync.dma_start(bt[:n], bsrc)
                xt = xp.tile([TILE_E, D], F32, tag="x")
                nc.sync.dma_start(xt[:n], xsrc)
                b3 = bt[:n].rearrange("p (i j) -> p i j", i=D)
                x3 = (xt[:n].rearrange("p (a j) -> p a j", a=1)
                      .to_broadcast((n, D, D)))
                if not pe or p_dt == F32:
                    nc.vector.tensor_mul(b3, b3, x3)
                    return bt, n
                pt = pp.tile([TILE_E, DD], p_dt, tag="p")
                nc.vector.tensor_mul(
                    pt[:n].rearrange("p (i j) -> p i j", i=D), b3, x3)
                return pt, n

            for entry in sched:
                kind = entry[0]
                if kind == "pe" and PSUM_TRICK:
                    # psum is [128, 64]: each bank-matmul's out AP repeats
                    # its 8 psum columns across the 64 streamed j columns
                    # (0-step dim) so has_written accumulation folds the
                    # j-sum.  Every matmul clears+writes its own 8-column
                    # range (start=True).
                    acc = ps.tile([TILE_E, D], F32, tag="acc")
                    for t in entry[1]:
                        pt, _ = load_and_mul(t, pe=True)
                        st = sp.tile([TILE_E, TILE_E], s_dt, tag="s")
                        nc.sync.dma_start(st[:], s_d[pe_i])
                        for bk in range(8):
                            out_ap = (acc[:, bk * 8:(bk + 1) * 8]
                                      .rearrange("p (i a) -> p i a", a=1)
                                      .to_broadcast((TILE_E, 8, D)))
                            nc.tensor.matmul(
                                out_ap, st[:],
                                pt[:, bk * 512:(bk + 1) * 512],
                                start=True, stop=True,
                                skip_group_check=True)
                        pe_i += 1
                    ot = op.tile([TILE_E, D], F32, tag="o")
                    nc.scalar.activation(ot[:], acc[:],
                                         mybir.ActivationFunctionType.Copy)
                    nc.sync.dma_start(out_pe[sb_i], ot[:])
                    sb_i += 1
                elif kind == "pe":
                    acc = ps.tile([TILE_E, DD], F32, tag="acc")
                    for ti, t in enumerate(entry[1]):
                        pt, _ = load_and_mul(t, pe=True)
                        st = sp.tile([TILE_E, TILE_E], s_dt, tag="s")
                        nc.sync.dma_start(st[:], s_d[pe_i])
                        first = ti == 0
                        last = ti == len(entry[1]) - 1
                        for bk in range(8):
                            nc.tensor.matmul(
                                acc[:, bk * 512:(bk + 1) * 512],
                                st[:],
                                pt[:, bk * 512:(bk + 1) * 512],
                                start=first, stop=last)
                        pe_i += 1
                    ot = op.tile([TILE_E, D], F32, tag="o")
                    nc.vector.reduce_sum(
                        ot[:], acc[:].rearrange("p (i j) -> p i j", i=D),
                        axis=mybir.AxisListType.X)
                    nc.sync.dma_start(out_pe[sb_i], ot[:])
                    sb_i += 1
                else:
                    is_tail = kind == "tail"
                    pt, n = load_and_mul(entry[1] if not is_tail else None,
                                         is_tail)
                    mt = mp.tile([TILE_E, D], F32, tag="m")
                    nc.vector.reduce_sum(
                        mt[:n], pt[:n].rearrange("p (i j) -> p i j", i=D),
                        axis=mybir.AxisListType.X)
                    if is_tail:
                        nc.sync.dma_start(msg_t[:], mt[:n])
                    else:
                        nc.sync.dma_start(msg_v[v_i], mt[:n])
                        v_i += 1

    nc.compile()
    return nc


def _get_nc():
    key = (NT_FULL, TAIL, K_PE, MM_DTYPE)
    if key not in _NC_CACHE:
        _NC_CACHE[key] = _build_nc()
    return _NC_CACHE[key]


def _segment_sum_sorted(messages, recv, n_atoms=N_ATOMS):
    starts = np.searchsorted(recv, np.arange(n_atoms))
    counts = np.diff(np.append(starts, len(recv)))
    out = np.add.reduceat(messages, np.minimum(starts, len(recv) - 1), axis=0)
    out[counts == 0] = 0
    return out


def _prepare(atom_matrix, bond_matrix, connectivity):
    atom_matrix = np.asarray(atom_matrix, dtype=np.float32)
    bond_matrix = np.asarray(bond_matrix, dtype=np.float32)
    connectivity = np.asarray(connectivity)

    recv = connectivity[:, 0].astype(np.int64)
    send = connectivity[:, 1].astype(np.int64)
    sent = np.ascontiguousarray(atom_matrix[send])       # (E, D)
    bond_flat = bond_matrix.reshape(N_EDGES, DD)

    sched = _schedule()
    s_np_dt = np.float32 if MM_DTYPE != "bf16" else None  # bf16 handled below

    in_maps = []
    meta = []  # per-core: dict with a0 (per sb), overflow edge indices
    for c in range(NCORES):
        lo = c * E_PER
        mid = lo + NT_FULL * TILE_E
        hi = lo + E_PER
        m = {
            "bond_m": bond_flat[lo:mid].reshape(NT_FULL, TILE_E, DD),
            "xs_m": sent[lo:mid].reshape(NT_FULL, TILE_E, D),
        }
        if TAIL:
            m["bond_t"] = bond_flat[mid:hi]
            m["xs_t"] = sent[mid:hi]

        # Build one-hot S for PE tiles + superblock base atoms.
        pe_tiles = [t for e in sched if e[0] == "pe" for t in e[1]]
        n_pe_tiles = len(pe_tiles)
        a0s = []
        ovf = []
        if n_pe_tiles:
            S = np.zeros((n_pe_tiles, TILE_E, TILE_E), np.float32)
            pe_i = 0
            for e in sched:
                if e[0] != "pe":
                    continue
                first_edge = lo + e[1][0] * TILE_E
                a0 = int(recv[first_edge])
                a0s.append(a0)
                for t in e[1]:
                    ge = lo + t * TILE_E          # global edge base
                    r = recv[ge:ge + TILE_E] - a0  # offsets in window
                    el = np.arange(TILE_E)
                    ok = r < TILE_E                # r >= 0 by sortedness
                    S[pe_i, el[ok], r[ok]] = 1.0
                    if not ok.all():
                        ovf.extend((ge + el[~ok]).tolist())
                    pe_i += 1
            if MM_DTYPE == "bf16":
                import ml_dtypes
                S = S.astype(ml_dtypes.bfloat16)
            m["s_onehot"] = S
        in_maps.append(m)
        meta.append({"a0s": a0s, "ovf": ovf})
    return in_maps, meta, recv, send, sent, bond_matrix, sched


def _combine(results, meta, recv, sent, bond_matrix, sched):
    final = np.zeros((N_ATOMS, D), np.float64)
    v_tiles = [e[1] for e in sched if e[0] == "v"]
    have_v = len(v_tiles) > 0 or TAIL

    if have_v:
        messages = np.zeros((N_EDGES, D), np.float32)
    for c, out in enumerate(results):
        lo = c * E_PER
        # PE window partials
        for si, a0 in enumerate(meta[c]["a0s"]):
            w = min(TILE_E, N_ATOMS - a0)
            final[a0:a0 + w] += out["out_pe"][si][:w]
        # V messages
        for vi, t in enumerate(v_tiles):
            ge = lo + t * TILE_E
            messages[ge:ge + TILE_E] = out["msg_v"][vi]
        if TAIL:
            mid = lo + NT_FULL * TILE_E
            messages[mid:lo + E_PER] = out["msg_t"]
        # overflow edges: recompute on host
        for ge in meta[c]["ovf"]:
            final[recv[ge]] += bond_matrix[ge] @ sent[ge]
    if have_v:
        final += _segment_sum_sorted(messages, recv)
    return final.astype(np.float32)


def kernel(atom_matrix, bond_matrix, connectivity):
    in_maps, meta, recv, send, sent, bond, sched = _prepare(
        atom_matrix, bond_matrix, connectivity)
    nc = _get_nc()

    os.environ["BASS_NEVER_TRACE"] = "1"  # no NTFF hook in this container
    res = bass_utils.run_bass_kernel_spmd(
        nc, in_maps, core_ids=list(range(NCORES)), trace=False)
    global LAST_RESULTS
    LAST_RESULTS = res

    return _combine(res.results, meta, recv, sent, bond, sched)


# ---------------------------------------------------------------------------
# Benchmark path: mirrors bass2jax.run_bass_via_pjrt's multi-core branch but
# pre-stages inputs on device so repeated calls measure device execution
# (plus per-call dispatch overhead, estimated via a null kernel).
# ---------------------------------------------------------------------------

def _make_runner(nc, n_cores=NCORES):
    import jax
    from jax.experimental.shard_map import shard_map
    from jax.sharding import Mesh, NamedSharding, PartitionSpec
    from concourse import bass2jax

    bass2jax.install_neuronx_cc_hook()
    partition_name = (nc.partition_id_tensor.name
                      if nc.partition_id_tensor else None)
    in_names, out_names, out_avals, zero_outs = [], [], [], []
    for alloc in nc.m.functions[0].allocations:
        if not isinstance(alloc, mybir.MemoryLocationSet):
            continue
        name = alloc.memorylocations[0].name
        if alloc.kind == "ExternalInput":
            if name != partition_name:
                in_names.append(name)
        elif alloc.kind == "ExternalOutput":
            import jax.core as jcore
            shape = tuple(alloc.tensor_shape)
            dtype = mybir.dt.np(alloc.dtype)
            out_names.append(name)
            out_avals.append(jcore.ShapedArray(shape, dtype))
            zero_outs.append(np.zeros(shape, dtype))
    n_params = len(in_names)
    n_outs = len(out_avals)
    in_names = in_names + out_names
    if partition_name is not None:
        in_names.append(partition_name)

    def _body(*args):
        operands = list(args)
        if partition_name is not None:
            operands.append(bass2jax.partition_id_tensor())
        outs = bass2jax._bass_exec_p.bind(
            *operands,
            out_avals=tuple(out_avals),
            in_names=tuple(in_names),
            out_names=tuple(out_names),
            lowering_input_output_aliases=(),
            sim_require_finite=True,
            sim_require_nnan=True,
            nc=nc,
        )
        return tuple(outs)

    devices = jax.devices()[:n_cores]
    mesh = Mesh(np.asarray(devices), ("core",))
    donate = tuple(range(n_params, n_params + n_outs))
    fn = jax.jit(
        shard_map(_body, mesh=mesh,
                  in_specs=(PartitionSpec("core"),) * (n_params + n_outs),
                  out_specs=(PartitionSpec("core"),) * n_outs,
                  check_rep=False),
        donate_argnums=donate, keep_unused=True)
    sharding = NamedSharding(mesh, PartitionSpec("core"))
    return dict(fn=fn, in_names=in_names[:n_params], out_names=out_names,
                zero_outs=zero_outs, sharding=sharding)


def _time_runner(runner, in_maps, iters):
    import jax
    import time as _time
    concat_in = [
        np.concatenate([np.asarray(m[name]) for m in in_maps], axis=0)
        for name in runner["in_names"]
    ]
    args = [jax.device_put(a, runner["sharding"]) for a in concat_in]
    zeros = [
        jax.device_put(np.zeros((NCORES * z.shape[0], *z.shape[1:]), z.dtype),
                       runner["sharding"])
        for z in runner["zero_outs"]
    ]
    outs = runner["fn"](*args, *zeros)
    jax.block_until_ready(outs)
    times = []
    for _ in range(iters):
        # The kernel writes every output element, so the previous outputs
        # are valid donation fodder — no host->device transfer per call.
        zeros = outs
        t0 = _time.perf_counter()
        outs = runner["fn"](*args, *zeros)
        jax.block_until_ready(outs)
        times.append(_time.perf_counter() - t0)
    return times


def _chain_runner(runner, in_maps, k_lo=5, k_hi=25, reps=3):
    """Chained async dispatch: slope of total time vs chain length isolates
    the per-call cost (device exec pipelined with ~1 ms client dispatch)."""
    import jax
    import time as _time
    concat_in = [
        np.concatenate([np.asarray(m[name]) for m in in_maps], axis=0)
        for name in runner["in_names"]
    ]
    args = [jax.device_put(a, runner["sharding"]) for a in concat_in]
    outs = [
        jax.device_put(np.zeros((NCORES * z.shape[0], *z.shape[1:]), z.dtype),
                       runner["sharding"])
        for z in runner["zero_outs"]
    ]
    outs = runner["fn"](*args, *outs)
    jax.block_until_ready(outs)

    def run_chain(k):
        nonlocal outs
        t0 = _time.perf_counter()
        o = outs
        for _ in range(k):
            o = runner["fn"](*args, *o)
        jax.block_until_ready(o)
        outs = o
        return _time.perf_counter() - t0

    slopes = []
    for _ in range(reps):
        t_lo = run_chain(k_lo)
        t_hi = run_chain(k_hi)
        slopes.append((t_hi - t_lo) / (k_hi - k_lo))
    return min(slopes)


def _make_chain_fn(nc, r, n_cores=NCORES):
    """Jit that executes the NEFF r times back-to-back on device (outputs of
    call i feed the donated output slots of call i+1). One host dispatch."""
    import jax
    from jax.experimental.shard_map import shard_map
    from jax.sharding import Mesh, NamedSharding, PartitionSpec
    from concourse import bass2jax

    bass2jax.install_neuronx_cc_hook()
    partition_name = (nc.partition_id_tensor.name
                      if nc.partition_id_tensor else None)
    in_names, out_names, out_avals, zero_outs = [], [], [], []
    for alloc in nc.m.functions[0].allocations:
        if not isinstance(alloc, mybir.MemoryLocationSet):
            continue
        name = alloc.memorylocations[0].name
        if alloc.kind == "ExternalInput":
            if name != partition_name:
                in_names.append(name)
        elif alloc.kind == "ExternalOutput":
            import jax.core as jcore
            shape = tuple(alloc.tensor_shape)
            dtype = mybir.dt.np(alloc.dtype)
            out_names.append(name)
            out_avals.append(jcore.ShapedArray(shape, dtype))
            zero_outs.append(np.zeros(shape, dtype))
    n_params = len(in_names)
    n_outs = len(out_avals)
    all_names = in_names + out_names
    if partition_name is not None:
        all_names.append(partition_name)

    def _body(*args):
        ins = list(args[:n_params])
        outs = list(args[n_params:])
        for _ in range(r):
            operands = ins + outs
            if partition_name is not None:
                operands.append(bass2jax.partition_id_tensor())
            outs = list(bass2jax._bass_exec_p.bind(
                *operands,
                out_avals=tuple(out_avals),
                in_names=tuple(all_names),
                out_names=tuple(out_names),
                lowering_input_output_aliases=(),
                sim_require_finite=True,
                sim_require_nnan=True,
                nc=nc,
            ))
        return tuple(outs)

    devices = jax.devices()[:n_cores]
    mesh = Mesh(np.asarray(devices), ("core",))
    donate = tuple(range(n_params, n_params + n_outs))
    fn = jax.jit(
        shard_map(_body, mesh=mesh,
                  in_specs=(PartitionSpec("core"),) * (n_params + n_outs),
                  out_specs=(PartitionSpec("core"),) * n_outs,
                  check_rep=False),
        donate_argnums=donate, keep_unused=True)
    sharding = NamedSharding(mesh, PartitionSpec("core"))
    return dict(fn=fn, in_names=in_names, zero_outs=zero_outs,
                sharding=sharding)


def _time_chain_inside(nc, in_maps, r_lo=2, r_hi=8, reps=4):
    """Per-execution device time from the slope over in-jit chain length."""
    import jax
    import time as _time
    res = {}
    for r in (r_lo, r_hi):
        runner = _make_chain_fn(nc, r)
        concat_in = [
            np.concatenate([np.asarray(m[name]) for m in in_maps], axis=0)
            for name in runner["in_names"]
        ]
        args = [jax.device_put(a, runner["sharding"]) for a in concat_in]
        zeros = [
            jax.device_put(
                np.zeros((NCORES * z.shape[0], *z.shape[1:]), z.dtype),
                runner["sharding"])
            for z in runner["zero_outs"]
        ]
        outs = runner["fn"](*args, *zeros)
        jax.block_until_ready(outs)
        times = []
        for _ in range(reps):
            zeros = outs
            t0 = _time.perf_counter()
            outs = runner["fn"](*args, *zeros)
            jax.block_until_ready(outs)
            times.append(_time.perf_counter() - t0)
        res[r] = min(times)
    return (res[r_hi] - res[r_lo]) / (r_hi - r_lo), res


def _build_null_nc():
    """Minimal kernel: one small DMA through SBUF, to estimate dispatch cost."""
    nc = bacc.Bacc("TRN2", target_bir_lowering=False, debug=False)
    xin = nc.dram_tensor("nul_in", [128, 16], F32, kind="ExternalInput")
    xout = nc.dram_tensor("nul_out", [128, 16], F32, kind="ExternalOutput")
    with tile.TileContext(nc) as tc:
        with tc.tile_pool(name="np_", bufs=1) as p:
            t = p.tile([128, 16], F32)
            nc.sync.dma_start(t[:], xin[:])
            nc.sync.dma_start(xout[:], t[:])
    nc.compile()
    return nc


def benchmark(atom_matrix, bond_matrix, connectivity, iters=20):
    in_maps, *_ = _prepare(atom_matrix, bond_matrix, connectivity)
    runner = _make_runner(_get_nc())
    times = _time_runner(runner, in_maps, iters)
    slope = _chain_runner(runner, in_maps)

    null_nc = _build_null_nc()
    null_runner = _make_runner(null_nc)
    null_maps = [{"nul_in": np.zeros((128, 16), np.float32)}
                 for _ in range(NCORES)]
    null_times = _time_runner(null_runner, null_maps, iters)
    null_slope = _chain_runner(null_runner, null_maps)

    t_min = min(times)
    t_null = min(null_times)
    return {
        "raw_min_ns": t_min * 1e9,
        "null_min_ns": t_null * 1e9,
        "sync_est_ns": max(t_min - t_null, 0.0) * 1e9,
        "slope_ns": slope * 1e9,
        "null_slope_ns": null_slope * 1e9,
        "hw_est_ns": slope * 1e9,
        "times_ns": [t * 1e9 for t in times],
    }

